# revision 12
# baseline (speedup 1.0000x reference)
"""Trainium2 Bass kernel for 16-head MultiHeadAttention (B=4, S=2048, H=1024).

Sharding: 8 cores = (batch b in 0..3) x (head-group g in 0..1).
Core (b, g) computes batch b, heads 8g..8g+7 (512 of the 1024 projected dims)
and produces a partial output out_partial.T [1024, 2048] (f32). Host sums the
two partials per batch and transposes back.

On-core layout is fully "transposed" (seq on the free dim everywhere):
  XT  [H=1024, S]   = x[b].T                      (bf16, host-prepped)
  QT/KT [D=512, S]  = W @ x.T + b                 (bf16, per-d-tile tiles)
  V   [S, 8*(64+1)] = x @ Wv_g.T + ones column per head (bf16)
  ST  [k, q] per (head, k-tile): scores^T          (psum f32)
  expST = exp(ST/8)                                (bf16)
  PV: psum[0:64] = V_h^T @ expST accumulated over k-tiles -> attnT (unnorm)
      psum[64]   = ones @ expST = softmax denominator (ones-column trick)
  attnT [d, q] normalized via DMA partition-broadcast + approx reciprocal
  outT  [o, q] = WoT_g^T @ attnT + bo_eff          (f32)

bv is folded away algebraically (softmax weights sum to 1): host adds Wo@bv
into bo_eff on the g=0 core.

Scheduling: the ACT engine's exp stream is the pacer during attention. The
PE work (scores + PV) underfills it, so projection matmul micro-groups are
interleaved as "filler" into the per-k-tile loop; PV lags scores by 2 k-tiles
so the PE never blocks on exp at its FIFO head.
"""

import os
from collections import deque
from contextlib import ExitStack

import numpy as np
import ml_dtypes

import concourse.bass as bass
import concourse.mybir as mybir
import concourse.tile as tile
from concourse import bacc

BF16 = mybir.dt.bfloat16
F32 = mybir.dt.float32
AF = mybir.ActivationFunctionType
ALU = mybir.AluOpType

P = 128
H = 1024          # model dim
NH = 16           # total heads
HD = 64           # head dim
G = 2             # tensor-parallel head groups
D = H // G        # 512 per-core projected dim
NHG = NH // G     # 8 heads per core
HT = H // P       # 8 h-tiles (contraction tiles for projections)
DT = D // P       # 4 d-tiles
VW = NHG * (HD + 1)  # 520 V width incl ones columns
MMW = 512         # matmul moving free dim
LAG = 2           # PV lags scores by this many k-tiles
FILL_EVERY = 5    # emit one filler projection group per this many k-tile steps


def emit(tc: tile.TileContext, S: int):
    nc = tc.nc
    ST_ = S // P                  # s-tiles == k-tiles
    QW = min(1024, S)             # q-chunk width (scores psum tile width)
    QH = S // QW                  # q chunks per head
    NQC = max(1, QW // MMW)       # matmuls per q chunk
    W = QW // NQC                 # matmul moving width

    xT = nc.declare_dram_parameter("xT", [H, S], BF16, isOutput=False)
    wqT = nc.declare_dram_parameter("wqT", [H, D], BF16, isOutput=False)
    wkT = nc.declare_dram_parameter("wkT", [H, D], BF16, isOutput=False)
    wvT = nc.declare_dram_parameter("wvT", [H, D], BF16, isOutput=False)
    woT = nc.declare_dram_parameter("woT", [D, H], BF16, isOutput=False)
    bqT = nc.declare_dram_parameter("bqT", [D], F32, isOutput=False)
    bkT = nc.declare_dram_parameter("bkT", [D], F32, isOutput=False)
    boE = nc.declare_dram_parameter("boE", [H], F32, isOutput=False)
    outT = nc.declare_dram_parameter("outT", [H, S], F32, isOutput=True)

    with ExitStack() as ctx:
        const = ctx.enter_context(tc.tile_pool(name="const", bufs=1))
        sps = ctx.enter_context(tc.tile_pool(name="sps", bufs=2, space="PSUM"))
        pvp = ctx.enter_context(tc.tile_pool(name="pvp", bufs=1, space="PSUM"))
        fil = ctx.enter_context(tc.tile_pool(name="fil", bufs=2, space="PSUM"))
        expp = ctx.enter_context(tc.tile_pool(name="expp", bufs=6))
        misc = ctx.enter_context(tc.tile_pool(name="misc", bufs=2))
        outp = ctx.enter_context(tc.tile_pool(name="outp", bufs=2))
        dramp = ctx.enter_context(tc.tile_pool(name="dramp", bufs=2, space="DRAM"))

        # ---- persistent SBUF tensors ----
        xts = [const.tile([P, S], BF16, tag=f"xt{i}", name=f"xt{i}") for i in range(HT)]
        wq = const.tile([P, HT, D], BF16, tag="wq")
        wk = const.tile([P, HT, D], BF16, tag="wk")
        wv = const.tile([P, HT, D], BF16, tag="wv")
        wo = const.tile([P, DT, H], BF16, tag="wo")
        bqv = const.tile([P, DT], F32, tag="bqv")
        bkv = const.tile([P, DT], F32, tag="bkv")
        bov = const.tile([P, HT], F32, tag="bov")
        qts = [const.tile([P, S], BF16, tag=f"qt{i}", name=f"qt{i}") for i in range(DT)]
        kts = [const.tile([P, S], BF16, tag=f"kt{i}", name=f"kt{i}") for i in range(DT)]
        vsb = const.tile([P, ST_, VW], BF16, tag="vsb")
        att = const.tile([P, DT, S], BF16, tag="att")

        # ---- input DMAs: weights first (small), then x per h-tile, so the
        # first projection groups start as soon as possible ----
        nc.sync.dma_start(wv[:], wvT[:].rearrange("(ht p) d -> p ht d", p=P))
        nc.sync.dma_start(wq[:], wqT[:].rearrange("(ht p) d -> p ht d", p=P))
        nc.sync.dma_start(wk[:], wkT[:].rearrange("(ht p) d -> p ht d", p=P))
        nc.sync.dma_start(bqv[:], bqT[:].rearrange("(t p) -> p t", p=P))
        nc.sync.dma_start(bkv[:], bkT[:].rearrange("(t p) -> p t", p=P))
        xv = xT[:].rearrange("(ht p) s -> p ht s", p=P)
        for ht in range(HT):
            nc.sync.dma_start(xts[ht][:], xv[:, ht, :])
        nc.sync.dma_start(wo[:], woT[:].rearrange("(dt p) o -> p dt o", p=P))
        nc.sync.dma_start(bov[:], boE[:].rearrange("(t p) -> p t", p=P))

        # ones columns of V (column HD within each head's 65-wide stripe)
        v4 = vsb[:].rearrange("p st (h c) -> p st h c", c=HD + 1)
        nc.vector.memset(v4[:, :, :, HD : HD + 1], 1.0)

        # ---- projection micro-groups ([128, 512] psum, 8 accumulating MMs) ----
        def qk_group(dst, w, bias, dt, sc):
            pt = fil.tile([P, MMW], F32, tag="fil")
            for ht in range(HT):
                nc.tensor.matmul(
                    pt[:],
                    lhsT=w[:, ht, dt * P : (dt + 1) * P],
                    rhs=xts[ht][:, sc * MMW : (sc + 1) * MMW],
                    start=(ht == 0),
                    stop=(ht == HT - 1),
                )
            nc.vector.tensor_scalar_add(
                dst[:, sc * MMW : (sc + 1) * MMW], pt[:], bias[:, dt : dt + 1]
            )

        def v_group(stile):
            pt = fil.tile([P, MMW], F32, tag="fil")
            for ht in range(HT):
                nc.tensor.matmul(
                    pt[:],
                    lhsT=xts[ht][:, stile * P : (stile + 1) * P],
                    rhs=wv[:, ht, :],
                    start=(ht == 0),
                    stop=(ht == HT - 1),
                )
            src = pt[:].rearrange("p (h c) -> p h c", h=NHG, c=HD)
            nc.vector.tensor_copy(v4[:, stile, :, 0:HD], src)

        NSC = S // MMW
        filler = deque()
        for dt in range(1, DT):
            for sc in range(NSC):
                filler.append(lambda dt=dt, sc=sc: qk_group(qts[dt], wq, bqv, dt, sc))
            for sc in range(NSC):
                filler.append(lambda dt=dt, sc=sc: qk_group(kts[dt], wk, bkv, dt, sc))

        # ---- output projection micro-group ([128, 512] psum) ----
        ot_view = outT[:].rearrange("(ot p) s -> p ot s", p=P)

        def o_group(ot, qcg):
            pt = fil.tile([P, MMW], F32, tag="fil")
            for dt in range(DT):
                nc.tensor.matmul(
                    pt[:],
                    lhsT=wo[:, dt, ot * P : (ot + 1) * P],
                    rhs=att[:, dt, qcg * MMW : (qcg + 1) * MMW],
                    start=(dt == 0),
                    stop=(dt == DT - 1),
                )
            ob = outp.tile([P, MMW], F32, tag="ob")
            nc.vector.tensor_scalar_add(ob[:], pt[:], bov[:, ot : ot + 1])
            nc.sync.dma_start(ot_view[:, ot, qcg * MMW : (qcg + 1) * MMW], ob[:])

        # ---- attention for one head, one q-chunk; filler interleaved ----
        def head_chunk(h, qh, fillq, fill_every):
            dt, off = h // 2, (h % 2) * HD
            ktile_sb, qtile_sb = kts[dt], qts[dt]
            pv = pvp.tile([P, QW], F32, tag="pvp")
            exs = {}
            for step in range(ST_ + LAG):
                if step < ST_:
                    kt_i = step
                    st = sps.tile([P, QW], F32, tag="sps")
                    for qc in range(NQC):
                        nc.tensor.matmul(
                            st[:, qc * W : (qc + 1) * W],
                            lhsT=ktile_sb[off : off + HD, kt_i * P : (kt_i + 1) * P],
                            rhs=qtile_sb[off : off + HD, qh * QW + qc * W : qh * QW + (qc + 1) * W],
                            start=True,
                            stop=True,
                        )
                    ex = expp.tile([P, QW], BF16, tag="ex")
                    nc.scalar.activation(ex[:], st[:], AF.Exp, scale=0.125)
                    exs[kt_i] = ex
                if step >= LAG:
                    kt_j = step - LAG
                    ex = exs.pop(kt_j)
                    for qc in range(NQC):
                        nc.tensor.matmul(
                            pv[0 : HD + 1, qc * W : (qc + 1) * W],
                            lhsT=vsb[:, kt_j, h * (HD + 1) : (h + 1) * (HD + 1)],
                            rhs=ex[:, qc * W : (qc + 1) * W],
                            start=(kt_j == 0),
                            stop=(kt_j == ST_ - 1),
                        )
                if fillq and fill_every and step % fill_every == fill_every - 1:
                    fillq.popleft()()
            # evict unnormalized + denom row (frees PSUM fast), then normalize
            # out-of-line via DMA partition-broadcast + approx reciprocal
            attu = misc.tile([HD + 1, QW], F32, tag="attu")
            nc.vector.tensor_copy(attu[:], pv[0 : HD + 1, :])
            dsc = dramp.tile([1, QW], F32, tag="dsc")
            nc.sync.dma_start(dsc[:], attu[HD : HD + 1, :])
            denr = misc.tile([HD, QW], F32, tag="denr")
            nc.sync.dma_start(denr[:], dsc[0:1, :].to_broadcast((HD, QW)))
            recr = misc.tile([HD, QW], F32, tag="recr")
            nc.vector.reciprocal_approx_fast(recr[:], denr[:])
            nc.vector.tensor_tensor(
                att[off : off + HD, dt, qh * QW : (qh + 1) * QW],
                attu[0:HD, :],
                recr[:],
                ALU.mult,
            )

        # ---- emission: minimal prefix, V and O projections stream as filler ----
        nv_pre = min(LAG + 2, ST_)
        for stile in range(nv_pre):
            v_group(stile)
        vfill = deque(
            (lambda stile=stile: v_group(stile)) for stile in range(nv_pre, ST_)
        )
        for sc in range(NSC):
            qk_group(qts[0], wq, bqv, 0, sc)
        for sc in range(NSC):
            qk_group(kts[0], wk, bkv, 0, sc)

        ofill = [
            deque((lambda ot=ot, qcg=qh * NQC + qc: o_group(ot, qcg))
                  for ot in range(HT) for qc in range(NQC))
            for qh in range(QH)
        ]

        for dt in range(DT):
            # make sure this d-tile's projections are emitted before its heads
            while filler and len(filler) > (DT - 1 - dt) * 2 * NSC:
                filler.popleft()()
            for hh in range(2):
                h = dt * 2 + hh
                for qh in range(QH):
                    if h == 0:
                        fq, ev = vfill, 1
                    elif h == NHG - 1 and qh == QH - 1 and QH > 1:
                        fq, ev = ofill[0], 1
                    elif dt < DT - 1:
                        fq, ev = filler, FILL_EVERY
                    else:
                        fq, ev = None, 0
                    head_chunk(h, qh, fq, ev)
        for q in (vfill, filler, *ofill):
            while q:
                q.popleft()()


def build_module(S: int = 2048):
    nc = bacc.Bacc("TRN2", target_bir_lowering=False, debug=False)
    with tile.TileContext(nc) as tc:
        emit(tc, S)
    nc.compile()
    return nc


def make_in_maps(x, Wq, bq, Wk, bk, Wv, bv, Wo, bo):
    """Host-side shard + layout prep. Core c = 2*b + g."""
    bf16 = ml_dtypes.bfloat16
    bo_eff = (bo + Wo.astype(np.float64) @ bv.astype(np.float64)).astype(np.float32)
    in_maps = []
    for b in range(4):
        xTb = np.ascontiguousarray(x[b].T).astype(bf16)
        for g in range(G):
            sl = slice(g * D, (g + 1) * D)
            in_maps.append(
                {
                    "xT": xTb,
                    "wqT": np.ascontiguousarray(Wq[sl, :].T).astype(bf16),
                    "wkT": np.ascontiguousarray(Wk[sl, :].T).astype(bf16),
                    "wvT": np.ascontiguousarray(Wv[sl, :].T).astype(bf16),
                    "woT": np.ascontiguousarray(Wo[:, sl].T).astype(bf16),
                    "bqT": np.ascontiguousarray(bq[sl]).astype(np.float32),
                    "bkT": np.ascontiguousarray(bk[sl]).astype(np.float32),
                    "boE": bo_eff if g == 0 else np.zeros(H, np.float32),
                }
            )
    return in_maps


_NC_CACHE = {}


def _get_module(S=2048):
    if S not in _NC_CACHE:
        _NC_CACHE[S] = build_module(S)
    return _NC_CACHE[S]


def kernel(x, Wq, bq, Wk, bk, Wv, bv, Wo, bo):
    from concourse.bass_utils import run_bass_kernel_spmd

    nc = _get_module(x.shape[1])
    in_maps = make_in_maps(x, Wq, bq, Wk, bk, Wv, bv, Wo, bo)
    trace = bool(int(os.environ.get("KERNEL_TRACE", "0")))
    res = run_bass_kernel_spmd(nc, in_maps, core_ids=list(range(8)), trace=trace)
    kernel.last_results = res
    out = np.empty((4, x.shape[1], H), np.float32)
    for b in range(4):
        acc = res.results[2 * b]["outT"] + res.results[2 * b + 1]["outT"]
        out[b] = acc.T
    return out


# revision 14
# speedup vs baseline: 1.0429x; 1.0429x over previous
"""Trainium2 Bass kernel for 16-head MultiHeadAttention (B=4, S=2048, H=1024).

Sharding: 8 cores = (batch b in 0..3) x (head-group g in 0..1).
Core (b, g) computes batch b, heads 8g..8g+7 (512 of the 1024 projected dims)
and produces a partial output out_partial.T [1024, 2048] (f32). Host sums the
two partials per batch and transposes back.

On-core layout is fully "transposed" (seq on the free dim everywhere):
  XT  [H=1024, S]   = x[b].T                      (bf16, host-prepped)
  QT/KT [D=512, S]  = W @ x.T + b                 (bf16, per-d-tile tiles)
  V   [S, 8*(64+1)] = x @ Wv_g.T + ones column per head (bf16)
  ST  [k, q] per (head, k-tile): scores^T          (psum f32)
  expST = exp(ST/8)                                (bf16)
  PV: psum[0:64] = V_h^T @ expST accumulated over k-tiles -> attnT (unnorm)
      psum[64]   = ones @ expST = softmax denominator (ones-column trick)
  attnT [d, q] normalized via DMA partition-broadcast + approx reciprocal
  outT  [o, q] = WoT_g^T @ attnT + bo_eff          (f32)

bv is folded away algebraically (softmax weights sum to 1): host adds Wo@bv
into bo_eff on the g=0 core.

Scheduling: the ACT engine's exp stream is the pacer during attention. The
PE work (scores + PV) underfills it, so projection matmul micro-groups are
interleaved as "filler" into the per-k-tile loop; PV lags scores by 2 k-tiles
so the PE never blocks on exp at its FIFO head.
"""

import os
from collections import deque
from contextlib import ExitStack

import numpy as np
import ml_dtypes

import concourse.bass as bass
import concourse.mybir as mybir
import concourse.tile as tile
from concourse import bacc

BF16 = mybir.dt.bfloat16
F32 = mybir.dt.float32
AF = mybir.ActivationFunctionType
ALU = mybir.AluOpType

P = 128
H = 1024          # model dim
NH = 16           # total heads
HD = 64           # head dim
G = 2             # tensor-parallel head groups
D = H // G        # 512 per-core projected dim
NHG = NH // G     # 8 heads per core
HT = H // P       # 8 h-tiles (contraction tiles for projections)
DT = D // P       # 4 d-tiles
VW = NHG * (HD + 1)  # 520 V width incl ones columns
MMW = 512         # matmul moving free dim
LAG = 3           # PV lags scores by this many k-tiles
FILL_EVERY = 5    # emit one filler projection group per this many k-tile steps


def emit(tc: tile.TileContext, S: int):
    nc = tc.nc
    ST_ = S // P                  # s-tiles == k-tiles
    QW = min(1024, S)             # q-chunk width (scores psum tile width)
    QH = S // QW                  # q chunks per head
    NQC = max(1, QW // MMW)       # matmuls per q chunk
    W = QW // NQC                 # matmul moving width

    xT = nc.declare_dram_parameter("xT", [H, S], BF16, isOutput=False)
    wqT = nc.declare_dram_parameter("wqT", [H, D], BF16, isOutput=False)
    wkT = nc.declare_dram_parameter("wkT", [H, D], BF16, isOutput=False)
    wvT = nc.declare_dram_parameter("wvT", [H, D], BF16, isOutput=False)
    woT = nc.declare_dram_parameter("woT", [D, H], BF16, isOutput=False)
    bqT = nc.declare_dram_parameter("bqT", [D], F32, isOutput=False)
    bkT = nc.declare_dram_parameter("bkT", [D], F32, isOutput=False)
    boE = nc.declare_dram_parameter("boE", [H], F32, isOutput=False)
    outT = nc.declare_dram_parameter("outT", [H, S], F32, isOutput=True)

    with ExitStack() as ctx:
        const = ctx.enter_context(tc.tile_pool(name="const", bufs=1))
        sps = ctx.enter_context(tc.tile_pool(name="sps", bufs=2, space="PSUM"))
        pvp = ctx.enter_context(tc.tile_pool(name="pvp", bufs=1, space="PSUM"))
        fil = ctx.enter_context(tc.tile_pool(name="fil", bufs=2, space="PSUM"))
        expp = ctx.enter_context(tc.tile_pool(name="expp", bufs=8))
        misc = ctx.enter_context(tc.tile_pool(name="misc", bufs=2))
        outp = ctx.enter_context(tc.tile_pool(name="outp", bufs=3))
        dramp = ctx.enter_context(tc.tile_pool(name="dramp", bufs=2, space="DRAM"))

        # ---- persistent SBUF tensors ----
        xts = [const.tile([P, S], BF16, tag=f"xt{i}", name=f"xt{i}") for i in range(HT)]
        wq = const.tile([P, HT, D], BF16, tag="wq")
        wk = const.tile([P, HT, D], BF16, tag="wk")
        wv = const.tile([P, HT, D], BF16, tag="wv")
        wo = const.tile([P, DT, H], BF16, tag="wo")
        bqv = const.tile([P, DT], F32, tag="bqv")
        bkv = const.tile([P, DT], F32, tag="bkv")
        bov = const.tile([P, HT], F32, tag="bov")
        qts = [const.tile([P, S], BF16, tag=f"qt{i}", name=f"qt{i}") for i in range(DT)]
        kts = [const.tile([P, S], BF16, tag=f"kt{i}", name=f"kt{i}") for i in range(DT)]
        vsb = const.tile([P, ST_, VW], BF16, tag="vsb")
        att = const.tile([P, DT, S], BF16, tag="att")

        # ---- input DMAs: weights first (small), then x per h-tile, so the
        # first projection groups start as soon as possible ----
        nc.sync.dma_start(wv[:], wvT[:].rearrange("(ht p) d -> p ht d", p=P))
        nc.sync.dma_start(wq[:], wqT[:].rearrange("(ht p) d -> p ht d", p=P))
        nc.sync.dma_start(wk[:], wkT[:].rearrange("(ht p) d -> p ht d", p=P))
        nc.sync.dma_start(bqv[:], bqT[:].rearrange("(t p) -> p t", p=P))
        nc.sync.dma_start(bkv[:], bkT[:].rearrange("(t p) -> p t", p=P))
        xv = xT[:].rearrange("(ht p) s -> p ht s", p=P)
        for ht in range(HT):
            nc.sync.dma_start(xts[ht][:], xv[:, ht, :])
        nc.sync.dma_start(wo[:], woT[:].rearrange("(dt p) o -> p dt o", p=P))
        nc.sync.dma_start(bov[:], boE[:].rearrange("(t p) -> p t", p=P))

        # ones columns of V (column HD within each head's 65-wide stripe)
        v4 = vsb[:].rearrange("p st (h c) -> p st h c", c=HD + 1)
        nc.vector.memset(v4[:, :, :, HD : HD + 1], 1.0)

        # ---- projection micro-groups ([128, 512] psum, 8 accumulating MMs) ----
        def qk_group(dst, w, bias, dt, sc):
            pt = fil.tile([P, MMW], F32, tag="fil")
            for ht in range(HT):
                nc.tensor.matmul(
                    pt[:],
                    lhsT=w[:, ht, dt * P : (dt + 1) * P],
                    rhs=xts[ht][:, sc * MMW : (sc + 1) * MMW],
                    start=(ht == 0),
                    stop=(ht == HT - 1),
                )
            nc.vector.tensor_scalar_add(
                dst[:, sc * MMW : (sc + 1) * MMW], pt[:], bias[:, dt : dt + 1]
            )

        def v_group(stile):
            pt = fil.tile([P, MMW], F32, tag="fil")
            for ht in range(HT):
                nc.tensor.matmul(
                    pt[:],
                    lhsT=xts[ht][:, stile * P : (stile + 1) * P],
                    rhs=wv[:, ht, :],
                    start=(ht == 0),
                    stop=(ht == HT - 1),
                )
            src = pt[:].rearrange("p (h c) -> p h c", h=NHG, c=HD)
            nc.vector.tensor_copy(v4[:, stile, :, 0:HD], src)

        NSC = S // MMW
        filler = deque()
        for dt in range(1, DT):
            for sc in range(NSC):
                filler.append(lambda dt=dt, sc=sc: qk_group(qts[dt], wq, bqv, dt, sc))
            for sc in range(NSC):
                filler.append(lambda dt=dt, sc=sc: qk_group(kts[dt], wk, bkv, dt, sc))

        # ---- output projection micro-group ([128, 512] psum) ----
        ot_view = outT[:].rearrange("(ot p) s -> p ot s", p=P)

        def o_group(ot, qcg):
            pt = fil.tile([P, MMW], F32, tag="fil")
            for dt in range(DT):
                nc.tensor.matmul(
                    pt[:],
                    lhsT=wo[:, dt, ot * P : (ot + 1) * P],
                    rhs=att[:, dt, qcg * MMW : (qcg + 1) * MMW],
                    start=(dt == 0),
                    stop=(dt == DT - 1),
                )
            ob = outp.tile([P, MMW], F32, tag="ob")
            nc.vector.tensor_scalar_add(ob[:], pt[:], bov[:, ot : ot + 1])
            nc.sync.dma_start(ot_view[:, ot, qcg * MMW : (qcg + 1) * MMW], ob[:])

        # ---- attention for one head, one q-chunk; filler interleaved ----
        def head_chunk(h, qh, fillq, fill_every):
            dt, off = h // 2, (h % 2) * HD
            ktile_sb, qtile_sb = kts[dt], qts[dt]
            pv = pvp.tile([P, QW], F32, tag="pvp")
            exs = {}
            for step in range(ST_ + LAG):
                if step < ST_:
                    kt_i = step
                    st = sps.tile([P, QW], F32, tag="sps")
                    for qc in range(NQC):
                        nc.tensor.matmul(
                            st[:, qc * W : (qc + 1) * W],
                            lhsT=ktile_sb[off : off + HD, kt_i * P : (kt_i + 1) * P],
                            rhs=qtile_sb[off : off + HD, qh * QW + qc * W : qh * QW + (qc + 1) * W],
                            start=True,
                            stop=True,
                        )
                    ex = expp.tile([P, QW], BF16, tag="ex")
                    nc.scalar.activation(ex[:], st[:], AF.Exp, scale=0.125)
                    exs[kt_i] = ex
                if step >= LAG:
                    kt_j = step - LAG
                    ex = exs.pop(kt_j)
                    for qc in range(NQC):
                        nc.tensor.matmul(
                            pv[0 : HD + 1, qc * W : (qc + 1) * W],
                            lhsT=vsb[:, kt_j, h * (HD + 1) : (h + 1) * (HD + 1)],
                            rhs=ex[:, qc * W : (qc + 1) * W],
                            start=(kt_j == 0),
                            stop=(kt_j == ST_ - 1),
                        )
                if fillq and fill_every and step % fill_every == fill_every - 1:
                    fillq.popleft()()
            # evict unnormalized + denom row (frees PSUM fast), then normalize
            # out-of-line via DMA partition-broadcast + approx reciprocal
            attu = misc.tile([HD + 1, QW], F32, tag="attu")
            nc.vector.tensor_copy(attu[:], pv[0 : HD + 1, :])
            dsc = dramp.tile([1, QW], F32, tag="dsc")
            nc.sync.dma_start(dsc[:], attu[HD : HD + 1, :])
            denr = misc.tile([HD, QW], F32, tag="denr")
            nc.sync.dma_start(denr[:], dsc[0:1, :].to_broadcast((HD, QW)))
            recr = misc.tile([HD, QW], F32, tag="recr")
            nc.vector.reciprocal_approx_fast(recr[:], denr[:])
            nc.vector.tensor_tensor(
                att[off : off + HD, dt, qh * QW : (qh + 1) * QW],
                attu[0:HD, :],
                recr[:],
                ALU.mult,
            )

        # ---- emission: minimal prefix, V and O projections stream as filler ----
        nv_pre = min(LAG + 2, ST_)
        for stile in range(nv_pre):
            v_group(stile)
        vfill = deque(
            (lambda stile=stile: v_group(stile)) for stile in range(nv_pre, ST_)
        )
        for sc in range(NSC):
            qk_group(qts[0], wq, bqv, 0, sc)
        for sc in range(NSC):
            qk_group(kts[0], wk, bkv, 0, sc)

        ofill = [
            deque((lambda ot=ot, qcg=qh * NQC + qc: o_group(ot, qcg))
                  for ot in range(HT) for qc in range(NQC))
            for qh in range(QH)
        ]

        # chunk-major: all heads at q-chunk 0, then all heads at q-chunk 1 with
        # the first chunk's output projection streaming as filler
        for qh in range(QH):
            for h in range(NHG):
                dt = h // 2
                # this d-tile's projections must be emitted before its heads
                if qh == 0:
                    while filler and len(filler) > max(0, (DT - 1 - dt)) * 2 * NSC:
                        filler.popleft()()
                if qh == 0 and h <= 1 and vfill:
                    fq, ev = vfill, 1
                elif qh == 0 and filler and dt < DT - 1:
                    fq, ev = filler, 4
                elif qh > 0 and ofill[qh - 1]:
                    fq, ev = ofill[qh - 1], 8
                else:
                    fq, ev = None, 0
                head_chunk(h, qh, fq, ev)
        for q in (vfill, filler, *ofill):
            while q:
                q.popleft()()


def build_module(S: int = 2048):
    nc = bacc.Bacc("TRN2", target_bir_lowering=False, debug=False)
    with tile.TileContext(nc) as tc:
        emit(tc, S)
    nc.compile()
    return nc


def make_in_maps(x, Wq, bq, Wk, bk, Wv, bv, Wo, bo):
    """Host-side shard + layout prep. Core c = 2*b + g."""
    bf16 = ml_dtypes.bfloat16
    bo_eff = (bo + Wo.astype(np.float64) @ bv.astype(np.float64)).astype(np.float32)
    in_maps = []
    for b in range(4):
        xTb = np.ascontiguousarray(x[b].T).astype(bf16)
        for g in range(G):
            sl = slice(g * D, (g + 1) * D)
            in_maps.append(
                {
                    "xT": xTb,
                    "wqT": np.ascontiguousarray(Wq[sl, :].T).astype(bf16),
                    "wkT": np.ascontiguousarray(Wk[sl, :].T).astype(bf16),
                    "wvT": np.ascontiguousarray(Wv[sl, :].T).astype(bf16),
                    "woT": np.ascontiguousarray(Wo[:, sl].T).astype(bf16),
                    "bqT": np.ascontiguousarray(bq[sl]).astype(np.float32),
                    "bkT": np.ascontiguousarray(bk[sl]).astype(np.float32),
                    "boE": bo_eff if g == 0 else np.zeros(H, np.float32),
                }
            )
    return in_maps


_NC_CACHE = {}


def _get_module(S=2048):
    if S not in _NC_CACHE:
        _NC_CACHE[S] = build_module(S)
    return _NC_CACHE[S]


def kernel(x, Wq, bq, Wk, bk, Wv, bv, Wo, bo):
    from concourse.bass_utils import run_bass_kernel_spmd

    nc = _get_module(x.shape[1])
    in_maps = make_in_maps(x, Wq, bq, Wk, bk, Wv, bv, Wo, bo)
    trace = bool(int(os.environ.get("KERNEL_TRACE", "0")))
    res = run_bass_kernel_spmd(nc, in_maps, core_ids=list(range(8)), trace=trace)
    kernel.last_results = res
    out = np.empty((4, x.shape[1], H), np.float32)
    for b in range(4):
        acc = res.results[2 * b]["outT"] + res.results[2 * b + 1]["outT"]
        out[b] = acc.T
    return out


# revision 15
# speedup vs baseline: 1.0468x; 1.0037x over previous
"""Trainium2 Bass kernel for 16-head MultiHeadAttention (B=4, S=2048, H=1024).

Sharding: 8 cores = (batch b in 0..3) x (head-group g in 0..1).
Core (b, g) computes batch b, heads 8g..8g+7 (512 of the 1024 projected dims)
and produces a partial output out_partial.T [1024, 2048] (f32). Host sums the
two partials per batch and transposes back.

On-core layout is fully "transposed" (seq on the free dim everywhere):
  XT  [H=1024, S]   = x[b].T                      (bf16, host-prepped)
  QT/KT [D=512, S]  = W @ x.T + b                 (bf16, per-d-tile tiles)
  V   [S, 8*(64+1)] = x @ Wv_g.T + ones column per head (bf16)
  ST  [k, q] per (head, k-tile): scores^T          (psum f32)
  expST = exp(ST/8)                                (bf16)
  PV: psum[0:64] = V_h^T @ expST accumulated over k-tiles -> attnT (unnorm)
      psum[64]   = ones @ expST = softmax denominator (ones-column trick)
  attnT [d, q] normalized via DMA partition-broadcast + approx reciprocal
  outT  [o, q] = WoT_g^T @ attnT + bo_eff          (f32)

bv is folded away algebraically (softmax weights sum to 1): host adds Wo@bv
into bo_eff on the g=0 core.

Scheduling: the ACT engine's exp stream is the pacer during attention. The
PE work (scores + PV) underfills it, so projection matmul micro-groups are
interleaved as "filler" into the per-k-tile loop; PV lags scores by 2 k-tiles
so the PE never blocks on exp at its FIFO head.
"""

import os
from collections import deque
from contextlib import ExitStack

import numpy as np
import ml_dtypes

import concourse.bass as bass
import concourse.mybir as mybir
import concourse.tile as tile
from concourse import bacc

BF16 = mybir.dt.bfloat16
F32 = mybir.dt.float32
AF = mybir.ActivationFunctionType
ALU = mybir.AluOpType

P = 128
H = 1024          # model dim
NH = 16           # total heads
HD = 64           # head dim
G = 2             # tensor-parallel head groups
D = H // G        # 512 per-core projected dim
NHG = NH // G     # 8 heads per core
HT = H // P       # 8 h-tiles (contraction tiles for projections)
DT = D // P       # 4 d-tiles
VW = NHG * (HD + 1)  # 520 V width incl ones columns
MMW = 512         # matmul moving free dim
LAG = 3           # PV lags scores by this many k-tiles
FILL_EVERY = 5    # emit one filler projection group per this many k-tile steps


def emit(tc: tile.TileContext, S: int):
    nc = tc.nc
    ST_ = S // P                  # s-tiles == k-tiles
    QW = min(1024, S)             # q-chunk width (scores psum tile width)
    QH = S // QW                  # q chunks per head
    NQC = max(1, QW // MMW)       # matmuls per q chunk
    W = QW // NQC                 # matmul moving width

    xT = nc.declare_dram_parameter("xT", [H, S], BF16, isOutput=False)
    wqT = nc.declare_dram_parameter("wqT", [H, D], BF16, isOutput=False)
    wkT = nc.declare_dram_parameter("wkT", [H, D], BF16, isOutput=False)
    wvT = nc.declare_dram_parameter("wvT", [H, D], BF16, isOutput=False)
    woT = nc.declare_dram_parameter("woT", [D, H], BF16, isOutput=False)
    bqT = nc.declare_dram_parameter("bqT", [D], F32, isOutput=False)
    bkT = nc.declare_dram_parameter("bkT", [D], F32, isOutput=False)
    boE = nc.declare_dram_parameter("boE", [H], F32, isOutput=False)
    outT = nc.declare_dram_parameter("outT", [H, S], F32, isOutput=True)

    with ExitStack() as ctx:
        const = ctx.enter_context(tc.tile_pool(name="const", bufs=1))
        sps = ctx.enter_context(tc.tile_pool(name="sps", bufs=2, space="PSUM"))
        pvp = ctx.enter_context(tc.tile_pool(name="pvp", bufs=1, space="PSUM"))
        fil = ctx.enter_context(tc.tile_pool(name="fil", bufs=2, space="PSUM"))
        expp = ctx.enter_context(tc.tile_pool(name="expp", bufs=8))
        misc = ctx.enter_context(tc.tile_pool(name="misc", bufs=2))
        outp = ctx.enter_context(tc.tile_pool(name="outp", bufs=3))
        dramp = ctx.enter_context(tc.tile_pool(name="dramp", bufs=2, space="DRAM"))

        # ---- persistent SBUF tensors ----
        xts = [const.tile([P, S], BF16, tag=f"xt{i}", name=f"xt{i}") for i in range(HT)]
        wq = const.tile([P, HT, D], BF16, tag="wq")
        wk = const.tile([P, HT, D], BF16, tag="wk")
        wv = const.tile([P, HT, D], BF16, tag="wv")
        wo = const.tile([P, DT, H], BF16, tag="wo")
        bqv = const.tile([P, DT], F32, tag="bqv")
        bkv = const.tile([P, DT], F32, tag="bkv")
        bov = const.tile([P, HT], F32, tag="bov")
        qts = [const.tile([P, S], BF16, tag=f"qt{i}", name=f"qt{i}") for i in range(DT)]
        kts = [const.tile([P, S], BF16, tag=f"kt{i}", name=f"kt{i}") for i in range(DT)]
        vsb = const.tile([P, ST_, VW], BF16, tag="vsb")
        att = const.tile([P, DT, S], BF16, tag="att")

        # ---- input DMAs: weights first (small), then x per h-tile, so the
        # first projection groups start as soon as possible ----
        nc.sync.dma_start(wv[:], wvT[:].rearrange("(ht p) d -> p ht d", p=P))
        nc.sync.dma_start(wq[:], wqT[:].rearrange("(ht p) d -> p ht d", p=P))
        nc.sync.dma_start(wk[:], wkT[:].rearrange("(ht p) d -> p ht d", p=P))
        nc.sync.dma_start(bqv[:], bqT[:].rearrange("(t p) -> p t", p=P))
        nc.sync.dma_start(bkv[:], bkT[:].rearrange("(t p) -> p t", p=P))
        xv = xT[:].rearrange("(ht p) s -> p ht s", p=P)
        for ht in range(HT):
            nc.sync.dma_start(xts[ht][:], xv[:, ht, :])
        nc.sync.dma_start(wo[:], woT[:].rearrange("(dt p) o -> p dt o", p=P))
        nc.sync.dma_start(bov[:], boE[:].rearrange("(t p) -> p t", p=P))

        # ones columns of V (column HD within each head's 65-wide stripe)
        v4 = vsb[:].rearrange("p st (h c) -> p st h c", c=HD + 1)
        nc.vector.memset(v4[:, :, :, HD : HD + 1], 1.0)

        # ---- projection micro-groups ([128, 512] psum, 8 accumulating MMs) ----
        def qk_group(dst, w, bias, dt, sc):
            pt = fil.tile([P, MMW], F32, tag="fil")
            for ht in range(HT):
                nc.tensor.matmul(
                    pt[:],
                    lhsT=w[:, ht, dt * P : (dt + 1) * P],
                    rhs=xts[ht][:, sc * MMW : (sc + 1) * MMW],
                    start=(ht == 0),
                    stop=(ht == HT - 1),
                )
            nc.vector.tensor_scalar_add(
                dst[:, sc * MMW : (sc + 1) * MMW], pt[:], bias[:, dt : dt + 1]
            )

        def v_group(stile):
            pt = fil.tile([P, MMW], F32, tag="fil")
            for ht in range(HT):
                nc.tensor.matmul(
                    pt[:],
                    lhsT=xts[ht][:, stile * P : (stile + 1) * P],
                    rhs=wv[:, ht, :],
                    start=(ht == 0),
                    stop=(ht == HT - 1),
                )
            src = pt[:].rearrange("p (h c) -> p h c", h=NHG, c=HD)
            nc.vector.tensor_copy(v4[:, stile, :, 0:HD], src)

        NSC = S // MMW
        filler = deque()
        for dt in range(1, DT):
            for sc in range(NSC):
                filler.append(lambda dt=dt, sc=sc: qk_group(qts[dt], wq, bqv, dt, sc))
            for sc in range(NSC):
                filler.append(lambda dt=dt, sc=sc: qk_group(kts[dt], wk, bkv, dt, sc))

        # ---- output projection micro-group ([128, 512] psum) ----
        ot_view = outT[:].rearrange("(ot p) s -> p ot s", p=P)

        def o_group(ot, qcg):
            pt = fil.tile([P, MMW], F32, tag="fil")
            for dt in range(DT):
                nc.tensor.matmul(
                    pt[:],
                    lhsT=wo[:, dt, ot * P : (ot + 1) * P],
                    rhs=att[:, dt, qcg * MMW : (qcg + 1) * MMW],
                    start=(dt == 0),
                    stop=(dt == DT - 1),
                )
            ob = outp.tile([P, MMW], F32, tag="ob")
            nc.vector.tensor_scalar_add(ob[:], pt[:], bov[:, ot : ot + 1])
            nc.sync.dma_start(ot_view[:, ot, qcg * MMW : (qcg + 1) * MMW], ob[:])

        # ---- attention for one head, one q-chunk; filler interleaved ----
        def head_chunk(h, qh, fillq, fill_every):
            dt, off = h // 2, (h % 2) * HD
            ktile_sb, qtile_sb = kts[dt], qts[dt]
            pv = pvp.tile([P, QW], F32, tag="pvp")
            exs = {}
            for step in range(ST_ + LAG):
                if step < ST_:
                    kt_i = step
                    st = sps.tile([P, QW], F32, tag="sps")
                    for qc in range(NQC):
                        nc.tensor.matmul(
                            st[:, qc * W : (qc + 1) * W],
                            lhsT=ktile_sb[off : off + HD, kt_i * P : (kt_i + 1) * P],
                            rhs=qtile_sb[off : off + HD, qh * QW + qc * W : qh * QW + (qc + 1) * W],
                            start=True,
                            stop=True,
                        )
                    ex = expp.tile([P, QW], BF16, tag="ex")
                    nc.scalar.activation(ex[:], st[:], AF.Exp, scale=0.125)
                    exs[kt_i] = ex
                if step >= LAG:
                    kt_j = step - LAG
                    ex = exs.pop(kt_j)
                    for qc in range(NQC):
                        nc.tensor.matmul(
                            pv[0 : HD + 1, qc * W : (qc + 1) * W],
                            lhsT=vsb[:, kt_j, h * (HD + 1) : (h + 1) * (HD + 1)],
                            rhs=ex[:, qc * W : (qc + 1) * W],
                            start=(kt_j == 0),
                            stop=(kt_j == ST_ - 1),
                        )
                if fillq and fill_every and step % fill_every == fill_every - 1:
                    fillq.popleft()()
            # evict unnormalized + denom row (frees PSUM fast), then normalize
            # out-of-line via DMA partition-broadcast + approx reciprocal
            attu = misc.tile([HD + 1, QW], F32, tag="attu")
            nc.vector.tensor_copy(attu[:], pv[0 : HD + 1, :])
            dsc = dramp.tile([1, QW], F32, tag="dsc")
            nc.sync.dma_start(dsc[:], attu[HD : HD + 1, :])
            denr = misc.tile([HD, QW], F32, tag="denr")
            nc.sync.dma_start(denr[:], dsc[0:1, :].to_broadcast((HD, QW)))
            recr = misc.tile([HD, QW], F32, tag="recr")
            nc.vector.reciprocal_approx_fast(recr[:], denr[:])
            nc.vector.tensor_tensor(
                att[off : off + HD, dt, qh * QW : (qh + 1) * QW],
                attu[0:HD, :],
                recr[:],
                ALU.mult,
            )

        # ---- emission: minimal prefix, V and O projections stream as filler ----
        nv_pre = min(LAG + 2, ST_)
        for stile in range(nv_pre):
            v_group(stile)
        vfill = deque(
            (lambda stile=stile: v_group(stile)) for stile in range(nv_pre, ST_)
        )
        for sc in range(NSC):
            qk_group(qts[0], wq, bqv, 0, sc)
        for sc in range(NSC):
            qk_group(kts[0], wk, bkv, 0, sc)

        ofill = [
            deque((lambda ot=ot, qcg=qh * NQC + qc: o_group(ot, qcg))
                  for ot in range(HT) for qc in range(NQC))
            for qh in range(QH)
        ]

        # chunk-major: all heads at q-chunk 0, then all heads at q-chunk 1 with
        # the first chunk's output projection streaming as filler
        for qh in range(QH):
            for h in range(NHG):
                dt = h // 2
                # this d-tile's projections must be emitted before its heads
                if qh == 0:
                    while filler and len(filler) > max(0, (DT - 1 - dt)) * 2 * NSC:
                        filler.popleft()()
                if qh == 0 and h <= 1 and vfill:
                    fq, ev = vfill, 1
                elif qh == 0 and filler and dt < DT - 1:
                    fq, ev = filler, 4
                elif qh > 0 and ofill[qh - 1]:
                    fq, ev = ofill[qh - 1], 8
                else:
                    fq, ev = None, 0
                head_chunk(h, qh, fq, ev)
        for q in (vfill, filler, *ofill):
            while q:
                q.popleft()()


def build_module(S: int = 2048):
    nc = bacc.Bacc("TRN2", target_bir_lowering=False, debug=False)
    with tile.TileContext(nc, pool_alloc_mode='queue') as tc:
        emit(tc, S)
    nc.compile()
    return nc


def make_in_maps(x, Wq, bq, Wk, bk, Wv, bv, Wo, bo):
    """Host-side shard + layout prep. Core c = 2*b + g."""
    bf16 = ml_dtypes.bfloat16
    bo_eff = (bo + Wo.astype(np.float64) @ bv.astype(np.float64)).astype(np.float32)
    in_maps = []
    for b in range(4):
        xTb = np.ascontiguousarray(x[b].T).astype(bf16)
        for g in range(G):
            sl = slice(g * D, (g + 1) * D)
            in_maps.append(
                {
                    "xT": xTb,
                    "wqT": np.ascontiguousarray(Wq[sl, :].T).astype(bf16),
                    "wkT": np.ascontiguousarray(Wk[sl, :].T).astype(bf16),
                    "wvT": np.ascontiguousarray(Wv[sl, :].T).astype(bf16),
                    "woT": np.ascontiguousarray(Wo[:, sl].T).astype(bf16),
                    "bqT": np.ascontiguousarray(bq[sl]).astype(np.float32),
                    "bkT": np.ascontiguousarray(bk[sl]).astype(np.float32),
                    "boE": bo_eff if g == 0 else np.zeros(H, np.float32),
                }
            )
    return in_maps


_NC_CACHE = {}


def _get_module(S=2048):
    if S not in _NC_CACHE:
        _NC_CACHE[S] = build_module(S)
    return _NC_CACHE[S]


def kernel(x, Wq, bq, Wk, bk, Wv, bv, Wo, bo):
    from concourse.bass_utils import run_bass_kernel_spmd

    nc = _get_module(x.shape[1])
    in_maps = make_in_maps(x, Wq, bq, Wk, bk, Wv, bv, Wo, bo)
    trace = bool(int(os.environ.get("KERNEL_TRACE", "0")))
    res = run_bass_kernel_spmd(nc, in_maps, core_ids=list(range(8)), trace=trace)
    kernel.last_results = res
    out = np.empty((4, x.shape[1], H), np.float32)
    for b in range(4):
        acc = res.results[2 * b]["outT"] + res.results[2 * b + 1]["outT"]
        out[b] = acc.T
    return out
